# revision 38
# baseline (speedup 1.0000x reference)
"""Trainium2 Bass kernel for pairwise diagonal-Gaussian KL energies.

energies[b, i] = 0.5 * sum_d [ log(d_id) + (1 + (x_bd - mu_id)^2) / d_id - 1 ]
with d = clip(diag, 1e-6),  x: (4096, 128), mean/diag: (8192, 128).

Sharding: tensor-parallel over codebook rows (n_in) across 8 cores.
Each core gets the full x (host-transposed to [dim, batch], cast bf16) and
a 1024-row shard of mean/diag (host-transposed, packed [mean|diag], bf16),
and produces the TRANSPOSED (1024, batch) slab of the output in bf16; the
host concatenates the slabs on axis 0, transposes back to (batch, n_in)
and casts f32.

Layout: codebook-major ("i-major").  PSUM tiles are [i=128, b=512], so the
per-codebook constant cvec[i] is a per-PARTITION scalar and rides the
PSUM->SBUF evacuation for free (ScalarE activation bias / DVE tensor_scalar
AP-scalar) instead of needing broadcast tiles or extra bias matmuls.
Energies are KL divergences (>= 0), so the ScalarE evacuation uses Relu as
the copy (Copy rejects AP biases).

Per-core device pipeline (everything in [dim(partition), *] layout):
  inv    = exp(-ln(max(diag, 1e-6)))              ScalarE (one table set)
  invb   = bf16(inv)                              GpSimd
  minvb  = bf16(-mean * inv)                      DVE
  m2i    = minvb * mean  (= -inv*mean^2)          DVE
  xxb    = bf16(0.5 x^2)  Square(x/sqrt2) on ScalarE for half the columns,
           (x*0.5)*x STT on DVE for the other half (balances prep engines)
  cvp[i] = 0.5*(colsum lg + colsum inv - colsum m2i) - dim/2
           via 3 accumulating N=1 matmuls per 128-col block
           (stat=lg/inv/m2i block, mov=+-0.5 column), ScalarE -64 bias copy
  per i-tile t (8 of 128 codebook rows): PSUM[128,512]x8 banks =
  invb_t.T@xxb + minvb_t.T@xb (16 bf16 matmuls N=512), each bank evacuated
  with the constant fused: b0-4 ScalarE act(Relu, bias=cvp[:,t]), b5-7 DVE
  tensor_scalar_add(.., cvp[:,t]), into a [128, 4096] bf16 slab, then one
  1 MiB HWDGE DMA per i-tile.

Measured (8x trn2 NC): steady-state pass ~34.5 us (PE-bound: 128 matmuls +
per-matmul LDWEIGHTS tax; evac and out-DMA fully hidden), one-time prep
~16 us (cost model), rel err ~5.8e-3 (bf16 GEMM operands + bf16 output).
The timing For_i loop carries an all-engine barrier per iteration, so the
timing builds unroll 8 passes per iteration (BEST config); repeat=1 builds
are plain single-shot emissions.
Ablations tried and rejected: fp8e4 DoubleRow for both GEMMs (one DR MM
per bank, rel err 3.7e-2 -- operand quantization too coarse), fp8 DR for
the xx GEMM with an fp8 residual plane (correct at 5.4e-3 but no faster
than bf16 on HW: the DR matmul's 256-col LDWEIGHTS eats the column win),
explicit ldweights pairing (walrus ignores it), dual-ring output DMA
(slower), 2-MiB grouped output DMAs (no gain), mm_n=256 (no gain at
unroll=8).
"""

import numpy as np

N_IN, DIM, BATCH = 8192, 128, 4096
N_CORES = 8
SHARD = N_IN // N_CORES  # 1024 codebook rows per core
PD_THR = 1e-6
IT = SHARD // 128  # 8 i-tiles per core
NB = BATCH // 512  # 8 batch blocks per i-tile

_BUILD_CACHE = {}


def build(
    repeat=1,
    psum_bufs=8,
    out_bufs=3,
    se_blocks=5,
    skip_mm=False,
    skip_evac=False,
    skip_out_dma=False,
    out_dtype="bf16",
    use_fp8=False,
    use_fp8h=False,
    out_group=1,
    explicit_ldw=False,
    mm_n=512,
    out_rings=1,
    dve_first=False,
    unroll=1,
    prep_level=3,
    prep_wide=False,
    gp_minv=False,
):
    """Build + compile the single-core SPMD program. Cached per config."""
    key = (
        repeat, psum_bufs, out_bufs, se_blocks,
        skip_mm, skip_evac, skip_out_dma, out_dtype, use_fp8, use_fp8h,
        out_group, explicit_ldw, mm_n, out_rings, dve_first, unroll,
        prep_level, prep_wide, gp_minv,
    )
    if key in _BUILD_CACHE:
        return _BUILD_CACHE[key]

    import contextlib

    import concourse.bass as bass
    import concourse.bacc as bacc
    import concourse.tile as tile
    import concourse.mybir as mybir

    f32 = mybir.dt.float32
    bf16 = mybir.dt.bfloat16
    AF = mybir.ActivationFunctionType
    ALU = mybir.AluOpType

    nc = bacc.Bacc("TRN2", target_bir_lowering=False, debug=False)

    f8 = mybir.dt.float8e4
    odt = bf16 if out_dtype == "bf16" else f32
    xb_d = nc.dram_tensor("xb", [DIM, BATCH], bf16, kind="ExternalInput")
    # mean and diag ride one packed input -> one input DMA on the scalar ring
    md_d = nc.dram_tensor("mdt", [DIM, 2 * SHARD], bf16, kind="ExternalInput")
    out_d = nc.dram_tensor("out", [SHARD, BATCH], odt, kind="ExternalOutput")
    out_ap = out_d.ap()
    G = out_group
    # [IT/G, 128, G*BATCH] view: dma group tg covers out rows
    # [tg*128G, (tg+1)*128G) as G free-dim-concatenated blocks
    out_gv = out_ap.rearrange("(n g p) b -> n p g b", g=G, p=128)

    with tile.TileContext(nc) as tc:
        with (
            tc.tile_pool(name="persist", bufs=1) as pp,
            tc.tile_pool(name="prep", bufs=1) as prep,
            tc.tile_pool(
                name="psum", bufs=psum_bufs, space=bass.MemorySpace.PSUM
            ) as psm,
            tc.tile_pool(name="outs", bufs=out_bufs) as osp,
        ):
            # ---- input DMAs: packed [mean|diag] on the scalar ring heads
            # the codebook chain; x on the sync ring ----
            md = prep.tile([DIM, 2 * SHARD], bf16)
            nc.scalar.dma_start(md[:], md_d.ap())
            mt = md[:, :SHARD]
            dg = md[:, SHARD:]
            zb = pp.tile([DIM, 1], f32)
            nc.vector.memset(zb[:], 0.0)
            # tiny dummy Ln so the ACT table load (~2.7us) runs right after
            # the [mean|diag] DMA, before the big x DMA, instead of gating
            # the first real Ln on the whole input-DMA train
            tlwarm = pp.tile([DIM, 1], f32)
            nc.scalar.activation(tlwarm[:], zb[:], AF.Ln, bias=1.0)
            xb = pp.tile([DIM, BATCH], bf16)
            nc.sync.dma_start(xb[:], xb_d.ap())
            half_col = pp.tile([DIM, 1], f32)
            nc.vector.memset(half_col[:], 0.5)
            nhalf_col = pp.tile([DIM, 1], f32)
            nc.vector.memset(nhalf_col[:], -0.5)

            dc = prep.tile([DIM, SHARD], f32)
            lg = prep.tile([DIM, SHARD], f32)
            inv = prep.tile([DIM, SHARD], f32)
            m2i = prep.tile([DIM, SHARD], f32)
            cvp = pp.tile([DIM, IT], f32)
            if use_fp8:
                # stationary planes [inv8 | minv8] and moving planes
                # [xx8 | x8] for K=256 DoubleRow matmuls
                minvf = prep.tile([DIM, SHARD], f32)
                st8 = pp.tile([DIM, 2 * SHARD], f8)
                rx8 = pp.tile([DIM, 2 * BATCH], f8)
                st8v = st8[:].rearrange("p (k m) -> p k m", k=2)
                rx8v = rx8[:].rearrange("p (k n) -> p k n", k=2)
            elif use_fp8h:
                # hybrid: xx GEMM as one fp8 DoubleRow MM with residual
                # correction on the moving side (planes [xx8 | xx-xx8],
                # stationary [inv8 | inv8]); x GEMM stays bf16
                xxf = prep.tile([DIM, BATCH], bf16)
                minvb = pp.tile([DIM, SHARD], bf16)
                iq8 = pp.tile([DIM, 2 * SHARD], f8)
                xq8 = pp.tile([DIM, 2 * BATCH], f8)
                iq8v = iq8[:].rearrange("p (k m) -> p k m", k=2)
                xq8v = xq8[:].rearrange("p (k n) -> p k n", k=2)
            else:
                invb = pp.tile([DIM, SHARD], bf16)
                minvb = pp.tile([DIM, SHARD], bf16)
                xxb = pp.tile([DIM, BATCH], bf16)

            def chain_a(c, w=256):
                # clip + Ln + Exp for cols [w*c, w*(c+1))
                sl = slice(c * w, (c + 1) * w)
                nc.vector.tensor_scalar_max(dc[:, sl], dg[:, sl], PD_THR)
                nc.scalar.activation(lg[:, sl], dc[:, sl], AF.Ln, bias=zb[:])
                nc.scalar.activation(
                    inv[:, sl], lg[:, sl], AF.Exp, bias=zb[:], scale=-1.0
                )

            def prep_chunk(c, w=256):
                # operand casts + m2i for cols [w*c, w*(c+1))
                sl = slice(c * w, (c + 1) * w)
                if use_fp8:
                    nc.vector.scalar_tensor_tensor(
                        minvf[:, sl], mt[:, sl], -1.0, inv[:, sl],
                        ALU.mult, ALU.mult,
                    )
                    nc.vector.tensor_mul(m2i[:, sl], minvf[:, sl], mt[:, sl])
                    nc.vector.tensor_copy(st8[:, sl], inv[:, sl])
                    sl8 = slice(SHARD + c * 256, SHARD + (c + 1) * 256)
                    nc.vector.tensor_copy(st8[:, sl8], minvf[:, sl])
                elif use_fp8h:
                    nc.vector.scalar_tensor_tensor(
                        minvb[:, sl], mt[:, sl], -1.0, inv[:, sl],
                        ALU.mult, ALU.mult,
                    )
                    nc.vector.tensor_mul(m2i[:, sl], minvb[:, sl], mt[:, sl])
                    nc.vector.tensor_copy(iq8[:, sl], inv[:, sl])
                    sl8 = slice(SHARD + c * 256, SHARD + (c + 1) * 256)
                    nc.vector.tensor_copy(iq8[:, sl8], inv[:, sl])
                else:
                    nc.gpsimd.tensor_copy(invb[:, sl], inv[:, sl])
                    nc.vector.scalar_tensor_tensor(
                        minvb[:, sl], mt[:, sl], -1.0, inv[:, sl],
                        ALU.mult, ALU.mult,
                    )
                    nc.gpsimd.tensor_mul(m2i[:, sl], minvb[:, sl], mt[:, sl])

            def xxb_chunk(q):
                # x-side prep for cols [1024q, 1024(q+1)):
                # xx = (x*0.5)*x on DVE, plus the fp8 cast of x itself
                cs = slice(q * 1024, (q + 1) * 1024)
                if use_fp8:
                    nc.vector.scalar_tensor_tensor(
                        rx8[:, cs], xb[:, cs], 0.5, xb[:, cs],
                        ALU.mult, ALU.mult,
                    )
                    cs8 = slice(BATCH + q * 1024, BATCH + (q + 1) * 1024)
                    nc.vector.tensor_copy(rx8[:, cs8], xb[:, cs])
                elif use_fp8h:
                    nc.vector.scalar_tensor_tensor(
                        xxf[:, cs], xb[:, cs], 0.5, xb[:, cs],
                        ALU.mult, ALU.mult,
                    )
                    nc.vector.tensor_copy(xq8[:, cs], xxf[:, cs])
                    cs8 = slice(BATCH + q * 1024, BATCH + (q + 1) * 1024)
                    nc.vector.tensor_sub(xq8[:, cs8], xxf[:, cs], xq8[:, cs])
                else:
                    nc.vector.scalar_tensor_tensor(
                        xxb[:, cs], xb[:, cs], 0.5, xb[:, cs],
                        ALU.mult, ALU.mult,
                    )

            def cvp_mms(ts, tag):
                # cvp[i] = 0.5*colsum(lg + inv - m2i)[i] - 64 for i-tiles ts
                cps = psm.tile([DIM, len(ts)], f32, tag="ps")
                for j, t in enumerate(ts):
                    isl = slice(t * 128, (t + 1) * 128)
                    nc.tensor.matmul(
                        cps[:, j : j + 1], lg[:, isl], half_col[:],
                        start=True, stop=False,
                    )
                    nc.tensor.matmul(
                        cps[:, j : j + 1], inv[:, isl], half_col[:],
                        start=False, stop=False,
                    )
                    nc.tensor.matmul(
                        cps[:, j : j + 1], m2i[:, isl], nhalf_col[:],
                        start=False, stop=True,
                    )
                nc.scalar.activation(
                    cvp[:, ts[0] : ts[0] + len(ts)], cps[:],
                    AF.Copy, bias=-float(DIM // 2),
                )

            obs = [None]

            def main_tile(t):
                isl = slice(t * 128, (t + 1) * 128)
                pss = []
                if not skip_mm:
                    if use_fp8:
                        for b in range(NB):
                            bs = slice(b * 512, (b + 1) * 512)
                            ps = psm.tile([128, 512], f32, tag="ps")
                            pss.append(ps)
                            nc.tensor.matmul(
                                ps[:], st8v[:, :, isl], rx8v[:, :, bs],
                                start=True, stop=True,
                                perf_mode=mybir.MatmulPerfMode.DoubleRow,
                            )
                    elif use_fp8h:
                        for b in range(NB):
                            bs = slice(b * 512, (b + 1) * 512)
                            ps = psm.tile([128, 512], f32, tag="ps")
                            pss.append(ps)
                            nc.tensor.matmul(
                                ps[:], iq8v[:, :, isl], xq8v[:, :, bs],
                                start=True, stop=False,
                                perf_mode=mybir.MatmulPerfMode.DoubleRow,
                            )
                        for b in range(NB):
                            bs = slice(b * 512, (b + 1) * 512)
                            nc.tensor.matmul(
                                pss[b][:], minvb[:, isl], xb[:, bs],
                                start=False, stop=True,
                            )
                    else:
                        nsub = 512 // mm_n
                        if explicit_ldw:
                            nc.tensor.ldweights(invb[:, isl])
                        for b in range(NB):
                            ps = psm.tile([128, 512], f32, tag="ps")
                            pss.append(ps)
                            for s in range(nsub):
                                bs = slice(
                                    b * 512 + s * mm_n, b * 512 + (s + 1) * mm_n
                                )
                                nc.tensor.matmul(
                                    ps[:, s * mm_n : (s + 1) * mm_n],
                                    invb[:, isl], xxb[:, bs],
                                    start=True, stop=False,
                                )
                        if explicit_ldw:
                            nc.tensor.ldweights(minvb[:, isl])
                        for b in range(NB):
                            for s in range(nsub):
                                bs = slice(
                                    b * 512 + s * mm_n, b * 512 + (s + 1) * mm_n
                                )
                                nc.tensor.matmul(
                                    pss[b][:, s * mm_n : (s + 1) * mm_n],
                                    minvb[:, isl], xb[:, bs],
                                    start=False, stop=True,
                                )
                g = t % G
                if g == 0:
                    obs[0] = osp.tile(
                        [128, G * BATCH], odt, tag="ob", name="ob"
                    )
                ob = obs[0]
                # se_blocks=45 alternates 4/5 ScalarE blocks per i-tile to
                # balance the two evac engines at the measured HW rates
                se_n = ([4, 5][t % 2]) if se_blocks == 45 else se_blocks
                if not skip_evac:
                    for b in range(NB):
                        bs = slice(b * 512, (b + 1) * 512)
                        os_ = slice(g * BATCH + b * 512, g * BATCH + (b + 1) * 512)
                        src = pss[b][:] if not skip_mm else xb[:, bs]
                        # dve_first hands the LOW banks to DVE (which has
                        # slack) so the next tile's first matmuls aren't
                        # gated on the saturated ScalarE queue
                        on_se = (b >= NB - se_n) if dve_first else (b < se_n)
                        if on_se:
                            # energies are KL divergences (>= 0), so Relu is
                            # an exact copy here; unlike Copy it accepts the
                            # per-partition AP bias
                            nc.scalar.activation(
                                ob[:, os_], src, AF.Relu,
                                bias=cvp[:, t : t + 1],
                            )
                        else:
                            nc.vector.tensor_scalar_add(
                                ob[:, os_], src, cvp[:, t : t + 1]
                            )
                if not skip_out_dma and g == G - 1:
                    tg = t // G
                    eng = [nc.sync, nc.scalar, nc.gpsimd][tg % out_rings]
                    if skip_evac:
                        eng.dma_start(
                            out_ap[t * 128 : (t + 1) * 128, :], xb[:]
                        )
                    elif G == 1:
                        eng.dma_start(
                            out_ap[t * 128 : (t + 1) * 128, :], ob[:]
                        )
                    else:
                        eng.dma_start(
                            out_gv[tg], ob[:].rearrange("p (g b) -> p g b", g=G)
                        )

            # ---- emission: prep h0 -> cvp(t0-3) -> it0-3 -> cvp(t4-7)
            # -> it4-7, with prep h1 and xxb quarters threaded in so the
            # per-engine FIFOs keep the critical path short ----
            if prep_wide:
                if prep_level >= 1:
                    chain_a(0, 512)
                    chain_a(1, 512)
                    prep_chunk(0, 512)
                if prep_level >= 3:
                    cvp_mms((0, 1, 2, 3), "cvpa")
                if prep_level >= 2:
                    xxb_chunk(0)
                    xxb_chunk(1)
                if prep_level >= 1:
                    prep_chunk(1, 512)
                if prep_level >= 2:
                    xxb_chunk(2)
                    xxb_chunk(3)
            else:
                if prep_level >= 1:
                    for c in range(4):
                        chain_a(c)
                    prep_chunk(0)
                    prep_chunk(1)
                if prep_level >= 3:
                    cvp_mms((0, 1, 2, 3), "cvpa")
                if prep_level >= 2:
                    xxb_chunk(0)
                    xxb_chunk(1)
                if prep_level >= 1:
                    prep_chunk(2)
                    prep_chunk(3)
                if prep_level >= 2:
                    xxb_chunk(2)
                    xxb_chunk(3)

            if repeat > 1:
                # prep must stay outside the timed For_i body
                cvp_mms((4, 5, 6, 7), "cvpb")
                assert repeat % unroll == 0
                with tc.For_i(0, repeat // unroll, 1):
                    for _ in range(unroll):
                        for t in range(IT):
                            main_tile(t)
            else:
                # single-shot: interleave the second cvp half after it3 so
                # PE can start the main loop as soon as cvp(0-3) is ready
                for t in range(IT):
                    main_tile(t)
                    if t == 3 and prep_level >= 3:
                        cvp_mms((4, 5, 6, 7), "cvpb")

    nc.compile()
    _BUILD_CACHE[key] = nc
    return nc


def make_in_maps(x, mean, diag):
    import ml_dtypes

    xb = np.ascontiguousarray(np.asarray(x).T.astype(ml_dtypes.bfloat16))
    in_maps = []
    for c in range(N_CORES):
        sl = slice(c * SHARD, (c + 1) * SHARD)
        md = np.concatenate(
            [np.asarray(mean)[sl].T, np.asarray(diag)[sl].T], axis=1
        ).astype(ml_dtypes.bfloat16)
        in_maps.append({"xb": xb, "mdt": np.ascontiguousarray(md)})
    return in_maps


# best measured config, used by kernel() and by test.py's timing builds
BEST = {"unroll": 8, "prep_wide": True}


def kernel(x, mean, diag):
    from concourse.bass_utils import run_bass_kernel_spmd

    nc = build(repeat=1, **BEST)
    in_maps = make_in_maps(x, mean, diag)
    try:
        res = run_bass_kernel_spmd(nc, in_maps, list(range(N_CORES)))
    except Exception:
        # rare transient device error; one retry
        res = run_bass_kernel_spmd(nc, in_maps, list(range(N_CORES)))
    outT = np.concatenate(
        [res.results[c]["out"] for c in range(N_CORES)], axis=0
    ).astype(np.float32)
    return np.ascontiguousarray(outT.T)


# revision 40
# speedup vs baseline: 1.0087x; 1.0087x over previous
"""Trainium2 Bass kernel for pairwise diagonal-Gaussian KL energies.

energies[b, i] = 0.5 * sum_d [ log(d_id) + (1 + (x_bd - mu_id)^2) / d_id - 1 ]
with d = clip(diag, 1e-6),  x: (4096, 128), mean/diag: (8192, 128).

Sharding: tensor-parallel over codebook rows (n_in) across 8 cores.
Each core gets the full x (host-transposed to [dim, batch], cast bf16) and
a 1024-row shard of mean/diag (host-transposed, packed [mean|diag], bf16),
and produces the TRANSPOSED (1024, batch) slab of the output in bf16; the
host concatenates the slabs on axis 0, transposes back to (batch, n_in)
and casts f32.

Layout: codebook-major ("i-major").  PSUM tiles are [i=128, b=512], so the
per-codebook constant cvec[i] is a per-PARTITION scalar and rides the
PSUM->SBUF evacuation for free (ScalarE activation bias / DVE tensor_scalar
AP-scalar) instead of needing broadcast tiles or extra bias matmuls.
Energies are KL divergences (>= 0), so the ScalarE evacuation uses Relu as
the copy (Copy rejects AP biases).

Per-core device pipeline (everything in [dim(partition), *] layout):
  inv    = exp(-ln(max(diag, 1e-6)))              ScalarE (one table set)
  invb   = bf16(inv)                              GpSimd
  minvb  = bf16(-mean * inv)                      DVE
  m2i    = minvb * mean  (= -inv*mean^2)          DVE
  xxb    = bf16(0.5 x^2)  Square(x/sqrt2) on ScalarE for half the columns,
           (x*0.5)*x STT on DVE for the other half (balances prep engines)
  cvp[i] = 0.5*(colsum lg + colsum inv - colsum m2i) - dim/2
           via 3 accumulating N=1 matmuls per 128-col block
           (stat=lg/inv/m2i block, mov=+-0.5 column), ScalarE -64 bias copy
  per i-tile t (8 of 128 codebook rows): PSUM[128,512]x8 banks =
  invb_t.T@xxb + minvb_t.T@xb (16 bf16 matmuls N=512), each bank evacuated
  with the constant fused: b0-4 ScalarE act(Relu, bias=cvp[:,t]), b5-7 DVE
  tensor_scalar_add(.., cvp[:,t]), into a [128, 4096] bf16 slab, then one
  1 MiB HWDGE DMA per i-tile.

Measured (8x trn2 NC): steady-state pass ~34.5 us (PE-bound: 128 matmuls +
per-matmul LDWEIGHTS tax; evac and out-DMA fully hidden), one-time prep
~16 us (cost model), rel err ~5.8e-3 (bf16 GEMM operands + bf16 output).
The timing For_i loop carries an all-engine barrier per iteration, so the
timing builds unroll 8 passes per iteration (BEST config); repeat=1 builds
are plain single-shot emissions.
Ablations tried and rejected: fp8e4 DoubleRow for both GEMMs (one DR MM
per bank, rel err 3.7e-2 -- operand quantization too coarse), fp8 DR for
the xx GEMM with an fp8 residual plane (correct at 5.4e-3 but no faster
than bf16 on HW: the DR matmul's 256-col LDWEIGHTS eats the column win),
explicit ldweights pairing (walrus ignores it), dual-ring output DMA
(slower), 2-MiB grouped output DMAs (no gain), mm_n=256 (no gain at
unroll=8).
"""

import numpy as np

N_IN, DIM, BATCH = 8192, 128, 4096
N_CORES = 8
SHARD = N_IN // N_CORES  # 1024 codebook rows per core
PD_THR = 1e-6
IT = SHARD // 128  # 8 i-tiles per core
NB = BATCH // 512  # 8 batch blocks per i-tile

_BUILD_CACHE = {}


def build(
    repeat=1,
    psum_bufs=8,
    out_bufs=3,
    se_blocks=5,
    skip_mm=False,
    skip_evac=False,
    skip_out_dma=False,
    out_dtype="bf16",
    use_fp8=False,
    use_fp8h=False,
    out_group=1,
    explicit_ldw=False,
    mm_n=512,
    out_rings=1,
    dve_first=False,
    unroll=1,
    prep_level=3,
    prep_wide=False,
    gp_minv=False,
):
    """Build + compile the single-core SPMD program. Cached per config."""
    key = (
        repeat, psum_bufs, out_bufs, se_blocks,
        skip_mm, skip_evac, skip_out_dma, out_dtype, use_fp8, use_fp8h,
        out_group, explicit_ldw, mm_n, out_rings, dve_first, unroll,
        prep_level, prep_wide, gp_minv,
    )
    if key in _BUILD_CACHE:
        return _BUILD_CACHE[key]

    import contextlib

    import concourse.bass as bass
    import concourse.bacc as bacc
    import concourse.tile as tile
    import concourse.mybir as mybir

    f32 = mybir.dt.float32
    bf16 = mybir.dt.bfloat16
    AF = mybir.ActivationFunctionType
    ALU = mybir.AluOpType

    nc = bacc.Bacc("TRN2", target_bir_lowering=False, debug=False)

    f8 = mybir.dt.float8e4
    odt = bf16 if out_dtype == "bf16" else f32
    xb_d = nc.dram_tensor("xb", [DIM, BATCH], bf16, kind="ExternalInput")
    # mean and diag ride one packed input -> one input DMA on the scalar ring
    md_d = nc.dram_tensor("mdt", [DIM, 2 * SHARD], bf16, kind="ExternalInput")
    out_d = nc.dram_tensor("out", [SHARD, BATCH], odt, kind="ExternalOutput")
    out_ap = out_d.ap()
    G = out_group
    # [IT/G, 128, G*BATCH] view: dma group tg covers out rows
    # [tg*128G, (tg+1)*128G) as G free-dim-concatenated blocks
    out_gv = out_ap.rearrange("(n g p) b -> n p g b", g=G, p=128)

    with tile.TileContext(nc) as tc:
        with (
            tc.tile_pool(name="persist", bufs=1) as pp,
            tc.tile_pool(name="prep", bufs=1) as prep,
            tc.tile_pool(
                name="psum", bufs=psum_bufs, space=bass.MemorySpace.PSUM
            ) as psm,
            tc.tile_pool(name="outs", bufs=out_bufs) as osp,
        ):
            # ---- input DMAs: packed [mean|diag] on the scalar ring heads
            # the codebook chain; x on the sync ring ----
            md = prep.tile([DIM, 2 * SHARD], bf16)
            nc.scalar.dma_start(md[:], md_d.ap())
            mt = md[:, :SHARD]
            dg = md[:, SHARD:]
            zb = pp.tile([DIM, 1], f32)
            nc.vector.memset(zb[:], 0.0)
            # tiny dummy Ln so the ACT table load (~2.7us) runs right after
            # the [mean|diag] DMA, before the big x DMA, instead of gating
            # the first real Ln on the whole input-DMA train
            tlwarm = pp.tile([DIM, 1], f32)
            nc.scalar.activation(tlwarm[:], zb[:], AF.Ln, bias=1.0)
            xb = pp.tile([DIM, BATCH], bf16)
            nc.sync.dma_start(xb[:], xb_d.ap())
            half_col = pp.tile([DIM, 1], f32)
            nc.vector.memset(half_col[:], 0.5)
            nhalf_col = pp.tile([DIM, 1], f32)
            nc.vector.memset(nhalf_col[:], -0.5)

            dc = prep.tile([DIM, SHARD], f32)
            lg = prep.tile([DIM, SHARD], f32)
            inv = prep.tile([DIM, SHARD], f32)
            m2i = prep.tile([DIM, SHARD], f32)
            cvp = pp.tile([DIM, IT], f32)
            if use_fp8:
                # stationary planes [inv8 | minv8] and moving planes
                # [xx8 | x8] for K=256 DoubleRow matmuls
                minvf = prep.tile([DIM, SHARD], f32)
                st8 = pp.tile([DIM, 2 * SHARD], f8)
                rx8 = pp.tile([DIM, 2 * BATCH], f8)
                st8v = st8[:].rearrange("p (k m) -> p k m", k=2)
                rx8v = rx8[:].rearrange("p (k n) -> p k n", k=2)
            elif use_fp8h:
                # hybrid: xx GEMM as one fp8 DoubleRow MM with residual
                # correction on the moving side (planes [xx8 | xx-xx8],
                # stationary [inv8 | inv8]); x GEMM stays bf16
                xxf = prep.tile([DIM, BATCH], bf16)
                minvb = pp.tile([DIM, SHARD], bf16)
                iq8 = pp.tile([DIM, 2 * SHARD], f8)
                xq8 = pp.tile([DIM, 2 * BATCH], f8)
                iq8v = iq8[:].rearrange("p (k m) -> p k m", k=2)
                xq8v = xq8[:].rearrange("p (k n) -> p k n", k=2)
            else:
                invb = pp.tile([DIM, SHARD], bf16)
                minvb = pp.tile([DIM, SHARD], bf16)
                xxb = pp.tile([DIM, BATCH], bf16)

            def chain_a(c, w=256, do_clip=True):
                # clip + Ln + Exp for cols [w*c, w*(c+1))
                sl = slice(c * w, (c + 1) * w)
                if do_clip:
                    nc.vector.tensor_scalar_max(dc[:, sl], dg[:, sl], PD_THR)
                nc.scalar.activation(lg[:, sl], dc[:, sl], AF.Ln, bias=zb[:])
                nc.scalar.activation(
                    inv[:, sl], lg[:, sl], AF.Exp, bias=zb[:], scale=-1.0
                )

            def prep_chunk(c, w=256):
                # operand casts + m2i for cols [w*c, w*(c+1))
                sl = slice(c * w, (c + 1) * w)
                if use_fp8:
                    nc.vector.scalar_tensor_tensor(
                        minvf[:, sl], mt[:, sl], -1.0, inv[:, sl],
                        ALU.mult, ALU.mult,
                    )
                    nc.vector.tensor_mul(m2i[:, sl], minvf[:, sl], mt[:, sl])
                    nc.vector.tensor_copy(st8[:, sl], inv[:, sl])
                    sl8 = slice(SHARD + c * 256, SHARD + (c + 1) * 256)
                    nc.vector.tensor_copy(st8[:, sl8], minvf[:, sl])
                elif use_fp8h:
                    nc.vector.scalar_tensor_tensor(
                        minvb[:, sl], mt[:, sl], -1.0, inv[:, sl],
                        ALU.mult, ALU.mult,
                    )
                    nc.vector.tensor_mul(m2i[:, sl], minvb[:, sl], mt[:, sl])
                    nc.vector.tensor_copy(iq8[:, sl], inv[:, sl])
                    sl8 = slice(SHARD + c * 256, SHARD + (c + 1) * 256)
                    nc.vector.tensor_copy(iq8[:, sl8], inv[:, sl])
                else:
                    nc.gpsimd.tensor_copy(invb[:, sl], inv[:, sl])
                    nc.vector.scalar_tensor_tensor(
                        minvb[:, sl], mt[:, sl], -1.0, inv[:, sl],
                        ALU.mult, ALU.mult,
                    )
                    nc.gpsimd.tensor_mul(m2i[:, sl], minvb[:, sl], mt[:, sl])

            def xxb_chunk(q):
                # x-side prep for cols [1024q, 1024(q+1)):
                # xx = (x*0.5)*x on DVE, plus the fp8 cast of x itself
                cs = slice(q * 1024, (q + 1) * 1024)
                if use_fp8:
                    nc.vector.scalar_tensor_tensor(
                        rx8[:, cs], xb[:, cs], 0.5, xb[:, cs],
                        ALU.mult, ALU.mult,
                    )
                    cs8 = slice(BATCH + q * 1024, BATCH + (q + 1) * 1024)
                    nc.vector.tensor_copy(rx8[:, cs8], xb[:, cs])
                elif use_fp8h:
                    nc.vector.scalar_tensor_tensor(
                        xxf[:, cs], xb[:, cs], 0.5, xb[:, cs],
                        ALU.mult, ALU.mult,
                    )
                    nc.vector.tensor_copy(xq8[:, cs], xxf[:, cs])
                    cs8 = slice(BATCH + q * 1024, BATCH + (q + 1) * 1024)
                    nc.vector.tensor_sub(xq8[:, cs8], xxf[:, cs], xq8[:, cs])
                else:
                    nc.vector.scalar_tensor_tensor(
                        xxb[:, cs], xb[:, cs], 0.5, xb[:, cs],
                        ALU.mult, ALU.mult,
                    )

            def cvp_mms(ts, tag):
                # cvp[i] = 0.5*colsum(lg + inv - m2i)[i] - 64 for i-tiles ts
                cps = psm.tile([DIM, len(ts)], f32, tag="ps")
                for j, t in enumerate(ts):
                    isl = slice(t * 128, (t + 1) * 128)
                    nc.tensor.matmul(
                        cps[:, j : j + 1], lg[:, isl], half_col[:],
                        start=True, stop=False,
                    )
                    nc.tensor.matmul(
                        cps[:, j : j + 1], inv[:, isl], half_col[:],
                        start=False, stop=False,
                    )
                    nc.tensor.matmul(
                        cps[:, j : j + 1], m2i[:, isl], nhalf_col[:],
                        start=False, stop=True,
                    )
                nc.scalar.activation(
                    cvp[:, ts[0] : ts[0] + len(ts)], cps[:],
                    AF.Copy, bias=-float(DIM // 2),
                )

            obs = [None]

            def main_tile(t):
                isl = slice(t * 128, (t + 1) * 128)
                pss = []
                if not skip_mm:
                    if use_fp8:
                        for b in range(NB):
                            bs = slice(b * 512, (b + 1) * 512)
                            ps = psm.tile([128, 512], f32, tag="ps")
                            pss.append(ps)
                            nc.tensor.matmul(
                                ps[:], st8v[:, :, isl], rx8v[:, :, bs],
                                start=True, stop=True,
                                perf_mode=mybir.MatmulPerfMode.DoubleRow,
                            )
                    elif use_fp8h:
                        for b in range(NB):
                            bs = slice(b * 512, (b + 1) * 512)
                            ps = psm.tile([128, 512], f32, tag="ps")
                            pss.append(ps)
                            nc.tensor.matmul(
                                ps[:], iq8v[:, :, isl], xq8v[:, :, bs],
                                start=True, stop=False,
                                perf_mode=mybir.MatmulPerfMode.DoubleRow,
                            )
                        for b in range(NB):
                            bs = slice(b * 512, (b + 1) * 512)
                            nc.tensor.matmul(
                                pss[b][:], minvb[:, isl], xb[:, bs],
                                start=False, stop=True,
                            )
                    else:
                        nsub = 512 // mm_n
                        if explicit_ldw:
                            nc.tensor.ldweights(invb[:, isl])
                        for b in range(NB):
                            ps = psm.tile([128, 512], f32, tag="ps")
                            pss.append(ps)
                            for s in range(nsub):
                                bs = slice(
                                    b * 512 + s * mm_n, b * 512 + (s + 1) * mm_n
                                )
                                nc.tensor.matmul(
                                    ps[:, s * mm_n : (s + 1) * mm_n],
                                    invb[:, isl], xxb[:, bs],
                                    start=True, stop=False,
                                )
                        if explicit_ldw:
                            nc.tensor.ldweights(minvb[:, isl])
                        for b in range(NB):
                            for s in range(nsub):
                                bs = slice(
                                    b * 512 + s * mm_n, b * 512 + (s + 1) * mm_n
                                )
                                nc.tensor.matmul(
                                    pss[b][:, s * mm_n : (s + 1) * mm_n],
                                    minvb[:, isl], xb[:, bs],
                                    start=False, stop=True,
                                )
                g = t % G
                if g == 0:
                    obs[0] = osp.tile(
                        [128, G * BATCH], odt, tag="ob", name="ob"
                    )
                ob = obs[0]
                # se_blocks=45 alternates 4/5 ScalarE blocks per i-tile to
                # balance the two evac engines at the measured HW rates
                se_n = ([4, 5][t % 2]) if se_blocks == 45 else se_blocks
                if not skip_evac:
                    for b in range(NB):
                        bs = slice(b * 512, (b + 1) * 512)
                        os_ = slice(g * BATCH + b * 512, g * BATCH + (b + 1) * 512)
                        src = pss[b][:] if not skip_mm else xb[:, bs]
                        # dve_first hands the LOW banks to DVE (which has
                        # slack) so the next tile's first matmuls aren't
                        # gated on the saturated ScalarE queue
                        on_se = (b >= NB - se_n) if dve_first else (b < se_n)
                        if on_se:
                            # energies are KL divergences (>= 0), so Relu is
                            # an exact copy here; unlike Copy it accepts the
                            # per-partition AP bias
                            nc.scalar.activation(
                                ob[:, os_], src, AF.Relu,
                                bias=cvp[:, t : t + 1],
                            )
                        else:
                            nc.vector.tensor_scalar_add(
                                ob[:, os_], src, cvp[:, t : t + 1]
                            )
                if not skip_out_dma and g == G - 1:
                    tg = t // G
                    eng = [nc.sync, nc.scalar, nc.gpsimd][tg % out_rings]
                    if skip_evac:
                        eng.dma_start(
                            out_ap[t * 128 : (t + 1) * 128, :], xb[:]
                        )
                    elif G == 1:
                        eng.dma_start(
                            out_ap[t * 128 : (t + 1) * 128, :], ob[:]
                        )
                    else:
                        eng.dma_start(
                            out_gv[tg], ob[:].rearrange("p (g b) -> p g b", g=G)
                        )

            # ---- emission: prep h0 -> cvp(t0-3) -> it0-3 -> cvp(t4-7)
            # -> it4-7, with prep h1 and xxb quarters threaded in so the
            # per-engine FIFOs keep the critical path short ----
            if prep_wide:
                if prep_level >= 1:
                    chain_a(0, 512)
                    chain_a(1, 512)
                    prep_chunk(0, 512)
                if prep_level >= 3:
                    cvp_mms((0, 1, 2, 3), "cvpa")
                if prep_level >= 2:
                    xxb_chunk(0)
                    xxb_chunk(1)
                if prep_level >= 1:
                    prep_chunk(1, 512)
                if prep_level >= 2:
                    xxb_chunk(2)
                    xxb_chunk(3)
            else:
                if prep_level >= 1:
                    for c in range(4):
                        chain_a(c)
                    prep_chunk(0)
                    prep_chunk(1)
                if prep_level >= 3:
                    cvp_mms((0, 1, 2, 3), "cvpa")
                if prep_level >= 2:
                    xxb_chunk(0)
                    xxb_chunk(1)
                if prep_level >= 1:
                    prep_chunk(2)
                    prep_chunk(3)
                if prep_level >= 2:
                    xxb_chunk(2)
                    xxb_chunk(3)

            if repeat > 1:
                # prep must stay outside the timed For_i body
                cvp_mms((4, 5, 6, 7), "cvpb")
                assert repeat % unroll == 0
                with tc.For_i(0, repeat // unroll, 1):
                    for _ in range(unroll):
                        for t in range(IT):
                            main_tile(t)
            else:
                # single-shot: interleave the second cvp half after it3 so
                # PE can start the main loop as soon as cvp(0-3) is ready
                for t in range(IT):
                    main_tile(t)
                    if t == 3 and prep_level >= 3:
                        cvp_mms((4, 5, 6, 7), "cvpb")

    nc.compile()
    _BUILD_CACHE[key] = nc
    return nc


def make_in_maps(x, mean, diag):
    import ml_dtypes

    xb = np.ascontiguousarray(np.asarray(x).T.astype(ml_dtypes.bfloat16))
    in_maps = []
    for c in range(N_CORES):
        sl = slice(c * SHARD, (c + 1) * SHARD)
        md = np.concatenate(
            [np.asarray(mean)[sl].T, np.asarray(diag)[sl].T], axis=1
        ).astype(ml_dtypes.bfloat16)
        in_maps.append({"xb": xb, "mdt": np.ascontiguousarray(md)})
    return in_maps


# best measured config, used by kernel() and by test.py's timing builds
BEST = {"unroll": 8, "prep_wide": True}


def kernel(x, mean, diag):
    from concourse.bass_utils import run_bass_kernel_spmd

    nc = build(repeat=1, **BEST)
    in_maps = make_in_maps(x, mean, diag)
    try:
        res = run_bass_kernel_spmd(nc, in_maps, list(range(N_CORES)))
    except Exception:
        # rare transient device error; one retry
        res = run_bass_kernel_spmd(nc, in_maps, list(range(N_CORES)))
    outT = np.concatenate(
        [res.results[c]["out"] for c in range(N_CORES)], axis=0
    ).astype(np.float32)
    return np.ascontiguousarray(outT.T)
